# revision 1
# baseline (speedup 1.0000x reference)
"""DCGRU classifier kernel for Trainium2 (8 NeuronCores, batch-data-parallel).

Layout strategy (per core, B_loc=4 batch items):
  - Activations kept FEATURE-major: tiles are (features, batch*node) so the
    recurrent state hT, gate outputs r/u, and candidate c need no transposes.
  - gconv reordered as  z@(W0-W2) + S@(z@W1) + (2S^2)@(z@W2); S^T and (2S^2)^T
    are host-precomputed so the two diffusion terms are independent.
  - Projections:  q = z@W  via matmul(lhsT=zT_b, rhs=W)  -> node-major psum.
    Diffusions:   [A@q]^T via matmul(lhsT=q_b, rhs=A^T)  -> feature-major psum.
  - Gate bias folded into ScalarE activation bias (per-partition in fm layout).
  - Per-step one-hot mask selects h2 at t == seq_len-1 (copy_predicated).
  - Final relu->fc->maxpool tail done on host (tiny).
"""

import os
import sys

import numpy as np

sys.path.insert(0, "/opt/trn_rl_repo")

import concourse.bass as bass
import concourse.bacc as bacc
import concourse.mybir as mybir
from concourse.bass_utils import run_bass_kernel_spmd
from concourse.tile import TileContext

B, T, N, DIN, U, C = 32, 256, 128, 16, 64, 4
NCORES = 8
BL = B // NCORES  # 4 batch items per core
BN = BL * N  # 512
F32 = mybir.dt.float32


# packed constant blob: every init constant in one DMA (keeps matmul sync
# waits within the LDWEIGHTS limit).  (row_count, col_offset, col_count)
def _blob_layout():
    lay = {}
    col = 0
    def seg(key, rows, cols):
        nonlocal col
        lay[key] = (rows, col, cols)
        col += cols
    seg("S_T", N, N)
    seg("S2_T", N, N)
    for l, D in ((0, DIN + U), (1, 2 * U)):
        seg((l, "g12"), D, 2 * 2 * U)
        seg((l, "g0"), D, 2 * U)
        seg((l, "c12"), D, 2 * U)
        seg((l, "c0"), D, U)
        for bkey in ("bgr", "bgu", "bgun", "bc"):
            seg((l, bkey), U, 1)
    return lay, col

_BLOB_LAYOUT, BLOB_COLS = _blob_layout()

_NC_CACHE = {}


def _build_nc(t_steps: int):
    nc = bacc.Bacc("TRN2")

    # ---- DRAM parameters (per core) ----
    xT_e = nc.declare_dram_parameter("xT", [t_steps, DIN, BN], F32, isOutput=False)
    blob_e = nc.declare_dram_parameter("blob", [N, BLOB_COLS], F32, isOutput=False)
    mask_e = nc.declare_dram_parameter("mask", [U, BL, t_steps], mybir.dt.int32, isOutput=False)
    out_e = nc.declare_dram_parameter("h2_last", [U, BN], F32, isOutput=True)

    with TileContext(nc) as tc:
        with (
            tc.tile_pool(name="singles", bufs=1) as singles,
            tc.tile_pool(name="sq", bufs=2) as sq_pool,
            tc.tile_pool(name="sqc", bufs=2) as sqc_pool,
            tc.tile_pool(name="sval", bufs=2) as sval_pool,
            tc.tile_pool(name="saux", bufs=2) as saux_pool,
            tc.tile_pool(name="pq", bufs=4, space="PSUM") as pq_pool,
            tc.tile_pool(name="pval", bufs=2, space="PSUM") as pval_pool,
            tc.tile_pool(name="pc", bufs=2, space="PSUM") as pc_pool,
        ):
            # ---- persistent tiles ----
            blob = singles.tile([N, BLOB_COLS], F32)
            nc.sync.dma_start(out=blob, in_=blob_e[:, :])
            def wv(key):
                rows, c0, cols = _BLOB_LAYOUT[key]
                return blob[0:rows, c0 : c0 + cols]
            st = wv("S_T")
            s2t = wv("S2_T")
            w = {k: wv(k) for k in _BLOB_LAYOUT if isinstance(k, tuple)}
            mask = singles.tile([U, BL, t_steps], mybir.dt.int32)
            nc.sync.dma_start(out=mask, in_=mask_e[:, :, :])

            # state tiles: zT / zcT per layer.  Partition slices must be
            # 32-aligned, so layer0 keeps h at rows 0:64 and x at rows 64:80
            # (weight rows are permuted on the host to match).
            #   layer0: z = [h1(64); x(16)]   layer1: z = [h1(64); h2(64)]
            zT0 = singles.tile([DIN + U, BN], F32)
            zcT0 = singles.tile([DIN + U, BN], F32)
            zT1 = singles.tile([2 * U, BN], F32)
            zcT1 = singles.tile([2 * U, BN], F32)
            acc = singles.tile([U, BN], F32)
            h2t = singles.tile([U, BN], F32)  # base-0 primary copy of h2 state
            nc.vector.memset(zT0[0:U, :], 0.0)
            nc.vector.memset(zT1[:, :], 0.0)
            nc.vector.memset(acc[:, :], 0.0)
            nc.vector.memset(h2t[:, :], 0.0)

            layers = (
                (0, DIN + U, zT0, zcT0, 0),  # (layer, D, zT, zcT, h_off)
                (1, 2 * U, zT1, zcT1, U),
            )

            for t in range(t_steps):
                nc.sync.dma_start(out=zT0[U : U + DIN, :], in_=xT_e[t])
                nc.sync.dma_start(out=zcT0[U : U + DIN, :], in_=xT_e[t])

                for l, D, zt, zct, ho in layers:
                    # ---- gate: q1,q2 projections (node-major psum) ----
                    pq1 = pq_pool.tile([N, BL, 2 * U], F32, tag="pq")
                    pq2 = pq_pool.tile([N, BL, 2 * U], F32, tag="pq")
                    for b in range(BL):
                        nc.tensor.matmul(
                            pq1[:, b, :],
                            lhsT=zt[:, b * N : (b + 1) * N],
                            rhs=w[l, "g12"][:, 0 : 2 * U],
                            start=True,
                            stop=True,
                        )
                        nc.tensor.matmul(
                            pq2[:, b, :],
                            lhsT=zt[:, b * N : (b + 1) * N],
                            rhs=w[l, "g12"][:, 2 * U : 4 * U],
                            start=True,
                            stop=True,
                        )
                    q1 = sq_pool.tile([N, BL, 2 * U], F32, tag="q1")
                    q2 = sq_pool.tile([N, BL, 2 * U], F32, tag="q2")
                    nc.scalar.copy(q1, pq1)
                    nc.vector.tensor_copy(q2, pq2)

                    # ---- gate: W0' term + diffusion terms -> fm psum ----
                    pval = pval_pool.tile([2 * U, BN], F32, tag="pval")
                    nc.tensor.matmul(pval, lhsT=w[l, "g0"], rhs=zt, start=True, stop=False)
                    for b in range(BL):
                        blk = pval[:, b * N : (b + 1) * N]
                        nc.tensor.matmul(
                            blk, lhsT=q1[:, b, :], rhs=st, start=False, stop=False,
                            skip_group_check=True,
                        )
                        nc.tensor.matmul(
                            blk, lhsT=q2[:, b, :], rhs=s2t, start=False,
                            stop=(b == BL - 1), skip_group_check=True,
                        )
                    h_prev = zt[0:U, :] if l == 0 else h2t[:, :]
                    r = sval_pool.tile([U, BN], F32, tag="r")
                    u = sval_pool.tile([U, BN], F32, tag="u")
                    um = saux_pool.tile([U, BN], F32, tag="um")
                    nc.scalar.activation(
                        r, pval[0:U, :], mybir.ActivationFunctionType.Sigmoid,
                        bias=w[l, "bgr"],
                    )
                    nc.scalar.activation(
                        u, pval[U : 2 * U, :], mybir.ActivationFunctionType.Sigmoid,
                        bias=w[l, "bgu"],
                    )
                    nc.scalar.activation(
                        um, pval[U : 2 * U, :], mybir.ActivationFunctionType.Sigmoid,
                        bias=w[l, "bgun"], scale=-1.0,
                    )

                    # ---- candidate path ----
                    # rhT = r * hT written straight into zcT's h-part
                    nc.vector.tensor_mul(zct[ho : ho + U, :], r, h_prev)
                    pqc = pq_pool.tile([N, BL, 2 * U], F32, tag="pq")
                    for b in range(BL):
                        nc.tensor.matmul(
                            pqc[:, b, :],
                            lhsT=zct[:, b * N : (b + 1) * N],
                            rhs=w[l, "c12"],
                            start=True,
                            stop=True,
                        )
                    qc = sqc_pool.tile([N, BL, 2 * U], F32, tag="qc")
                    nc.scalar.copy(qc, pqc)
                    pc = pc_pool.tile([U, BN], F32, tag="pc")
                    nc.tensor.matmul(pc, lhsT=w[l, "c0"], rhs=zct, start=True, stop=False)
                    for b in range(BL):
                        blk = pc[:, b * N : (b + 1) * N]
                        nc.tensor.matmul(
                            blk, lhsT=qc[:, b, 0:U], rhs=st, start=False, stop=False,
                            skip_group_check=True,
                        )
                        nc.tensor.matmul(
                            blk, lhsT=qc[:, b, U : 2 * U], rhs=s2t, start=False,
                            stop=(b == BL - 1), skip_group_check=True,
                        )
                    c = sval_pool.tile([U, BN], F32, tag="c")
                    nc.scalar.activation(
                        c, pc, mybir.ActivationFunctionType.Tanh, bias=w[l, "bc"]
                    )

                    # ---- state update: h' = u*h + (1-u)*c = u*h + um*c ----
                    t1 = saux_pool.tile([U, BN], F32, tag="t1")
                    nc.gpsimd.tensor_tensor(t1, u, h_prev, op=mybir.AluOpType.mult)
                    m = saux_pool.tile([U, BN], F32, tag="m")
                    nc.vector.tensor_mul(m, c, um)
                    h_new = zt[0:U, :] if l == 0 else h2t[:, :]
                    nc.vector.tensor_add(h_new, m, t1)

                    if l == 0:
                        # h1_t feeds layer1's x-part (both zT1 and zcT1)
                        nc.gpsimd.tensor_copy(zT1[0:U, :], zT0[0:U, :])
                        nc.gpsimd.tensor_copy(zcT1[0:U, :], zT0[0:U, :])
                    else:
                        nc.gpsimd.tensor_copy(zT1[U : 2 * U, :], h2t)
                        # select h2_t into acc where t == seq_len-1
                        nc.vector.copy_predicated(
                            acc[:, :].rearrange("p (b n) -> p b n", b=BL),
                            mask[:, :, t : t + 1].to_broadcast([U, BL, N]),
                            h2t[:, :].rearrange("p (b n) -> p b n", b=BL),
                        )

            nc.sync.dma_start(out=out_e[:, :], in_=acc)

    nc.compile()
    return nc


def _prep_shared(support, W0_gate, W0_cand, W1_gate, W1_cand,
                 b0_gate, b0_cand, b1_gate, b1_cand):
    f = np.float32
    S = np.asarray(support, f)
    seg = {
        "S_T": np.ascontiguousarray(S.T),
        "S2_T": np.ascontiguousarray((2.0 * (S @ S)).T),
    }
    for l, (Wg, Wc, bg, bc) in enumerate(
        ((W0_gate, W0_cand, b0_gate, b0_cand), (W1_gate, W1_cand, b1_gate, b1_cand))
    ):
        Wg = np.asarray(Wg, f)
        Wc = np.asarray(Wc, f)
        g = [Wg[m::3] for m in range(3)]
        c = [Wc[m::3] for m in range(3)]
        if l == 0:
            # device z-layout for layer0 is [h(64); x(16)]
            perm = np.concatenate([np.arange(DIN, DIN + U), np.arange(DIN)])
            g = [gm[perm] for gm in g]
            c = [cm[perm] for cm in c]
        seg[(l, "g12")] = np.concatenate([g[1], g[2]], axis=1)
        seg[(l, "g0")] = g[0] - g[2]
        seg[(l, "c12")] = np.concatenate([c[1], c[2]], axis=1)
        seg[(l, "c0")] = c[0] - c[2]
        bg = np.asarray(bg, f).reshape(-1)
        seg[(l, "bgr")] = bg[:U].reshape(U, 1)
        seg[(l, "bgu")] = bg[U:].reshape(U, 1)
        seg[(l, "bgun")] = (-bg[U:]).reshape(U, 1)
        seg[(l, "bc")] = np.asarray(bc, f).reshape(U, 1)
    blob = np.zeros((N, BLOB_COLS), f)
    for key, (rows, c0, cols) in _BLOB_LAYOUT.items():
        a = seg[key]
        assert a.shape == (rows, cols), (key, a.shape, rows, cols)
        blob[:rows, c0 : c0 + cols] = a
    return {"blob": blob}


def run_cores(inputs, t_steps=T, trace=False):
    """Build in_maps, run the SPMD kernel, return per-core h2_last plus results."""
    input_seq = np.asarray(inputs["input_seq"], np.float32)
    seq_lengths = np.asarray(inputs["seq_lengths"]).astype(np.int64)
    shared = _prep_shared(
        inputs["support"], inputs["W0_gate"], inputs["W0_cand"],
        inputs["W1_gate"], inputs["W1_cand"],
        inputs["b0_gate"], inputs["b0_cand"], inputs["b1_gate"], inputs["b1_cand"],
    )
    in_maps = []
    for k in range(NCORES):
        xs = input_seq[k * BL : (k + 1) * BL, :t_steps]  # (BL, t, N, DIN)
        xT = np.ascontiguousarray(
            np.transpose(xs, (1, 3, 0, 2)).reshape(t_steps, DIN, BN)
        )
        idx = np.minimum(seq_lengths[k * BL : (k + 1) * BL] - 1, t_steps - 1)
        mask = np.zeros((BL, t_steps), np.int32)
        mask[np.arange(BL), idx] = 1
        m = np.broadcast_to(mask[None], (U, BL, t_steps))
        in_maps.append(
            dict(shared, xT=xT, mask=np.ascontiguousarray(m))
        )
    if t_steps not in _NC_CACHE:
        _NC_CACHE[t_steps] = _build_nc(t_steps)
    nc = _NC_CACHE[t_steps]
    res = run_bass_kernel_spmd(nc, in_maps, list(range(NCORES)), trace=trace)
    return res


def finish_host(results, inputs):
    """Host tail: relu -> fc -> node max-pool."""
    W_fc = np.asarray(inputs["W_fc"], np.float32)
    b_fc = np.asarray(inputs["b_fc"], np.float32)
    out = np.empty((B, C), np.float32)
    for k in range(NCORES):
        h2 = results[k]["h2_last"]  # (U, BL*N) feature-major
        for b in range(BL):
            blk = h2[:, b * N : (b + 1) * N].T  # (N, U)
            logits = np.maximum(blk, 0.0) @ W_fc + b_fc  # (N, C)
            out[k * BL + b] = logits.max(axis=0)
    return out


def kernel(**inputs):
    res = run_cores(inputs, t_steps=T)
    return finish_host(res.results, inputs)



# revision 6
# speedup vs baseline: 1.9971x; 1.9971x over previous
"""DCGRU classifier kernel for Trainium2 (8 NeuronCores, batch-data-parallel).

v2 layout strategy (per core, B_loc=4 batch items):
  - All matmul operands bf16 (4x PE throughput vs fp32); PSUM accumulates fp32.
  - Activations FEATURE-major: tiles are (features, batch*node) so the
    recurrent state, gates and candidate need no transposes.
  - gconv reordered as  z@(W0-W2) + S@(z@W1) + (2S^2)@(z@W2); S^T and (2S^2)^T
    are host-precomputed so the two diffusion terms are independent.
  - Gate projections combined: z@[W1|W2] -> (N, 256) per batch, one matmul.
  - One sigmoid for [r|u] (bias per-partition, stacked), no 1-u activation:
    state update is h' = c + u*(h-c)  (3 DVE ops).
  - h2 state ping-pongs between two tiles; every step h2_t is DMA'd to DRAM,
    host picks t = seq_len-1 per item (replaces in-loop predicated select).
  - t_steps = max(seq_lengths) (host-side, steps beyond it are never read).
  - Final relu->fc->maxpool tail done on host (tiny).
"""

import sys

import numpy as np
import ml_dtypes

sys.path.insert(0, "/opt/trn_rl_repo")

import concourse.bass as bass
import concourse.bacc as bacc
import concourse.mybir as mybir
from concourse.bass_utils import run_bass_kernel_spmd
from concourse.tile import TileContext

B, T, N, DIN, U, C = 32, 256, 128, 16, 64, 4
NCORES = 8
BL = B // NCORES  # 4 batch items per core
BN = BL * N  # 512
F32 = mybir.dt.float32
BF16 = mybir.dt.bfloat16
BF16_NP = ml_dtypes.bfloat16


# packed bf16 constant blob: (row_count, col_offset, col_count)
def _blob_layout():
    lay = {}
    col = 0

    def seg(key, rows, cols):
        nonlocal col
        lay[key] = (rows, col, cols)
        col += cols

    seg("S_T", N, N)
    seg("S2_T", N, N)
    for l, D in ((0, DIN + U), (1, 2 * U)):
        seg((l, "g12"), D, 4 * U)
        seg((l, "g0"), D, 2 * U)
        seg((l, "c12"), D, 2 * U)
        seg((l, "c0"), D, U)
    return lay, col


_BLOB_LAYOUT, BLOB_COLS = _blob_layout()

_NC_CACHE = {}


def _build_nc(t_steps: int):
    nc = bacc.Bacc("TRN2")

    xT_e = nc.declare_dram_parameter("xT", [t_steps, DIN, BN], BF16, isOutput=False)
    blob_e = nc.declare_dram_parameter("blob", [N, BLOB_COLS], BF16, isOutput=False)
    bias_e = nc.declare_dram_parameter("bias", [N, 4], F32, isOutput=False)
    h2seq_e = nc.declare_dram_parameter("h2seq", [t_steps, U, BN], BF16, isOutput=True)

    with TileContext(nc) as tc:
        with (
            tc.tile_pool(name="singles", bufs=1) as singles,
            tc.tile_pool(name="sq", bufs=2) as sq_pool,
            tc.tile_pool(name="sval", bufs=2) as sval_pool,
            tc.tile_pool(name="pq", bufs=2, space="PSUM") as pq_pool,
            tc.tile_pool(name="pqc", bufs=2, space="PSUM") as pqc_pool,
            tc.tile_pool(name="pval", bufs=2, space="PSUM") as pval_pool,
            tc.tile_pool(name="pc", bufs=2, space="PSUM") as pc_pool,
        ):
            # ---- persistent tiles ----
            blob = singles.tile([N, BLOB_COLS], BF16)
            nc.sync.dma_start(out=blob, in_=blob_e[:, :])
            bias = singles.tile([N, 4], F32)
            nc.sync.dma_start(out=bias, in_=bias_e[:, :])

            def wv(key):
                rows, c0, cols = _BLOB_LAYOUT[key]
                return blob[0:rows, c0 : c0 + cols]

            st = wv("S_T")
            s2t = wv("S2_T")
            w = {k: wv(k) for k in _BLOB_LAYOUT if isinstance(k, tuple)}

            # state tiles.  layer0 z-layout: [h1(0:U); x(U:U+DIN)].
            # layer1 z-layout: [h2(0:U); h1(U:2U)] (h2 first so its elementwise
            # ops stay at partition 0), ping-pong pair.
            zT0 = singles.tile([DIN + U, BN], BF16)
            zcT0 = singles.tile([DIN + U, BN], BF16)
            zT1_0 = singles.tile([2 * U, BN], BF16)
            zT1_1 = singles.tile([2 * U, BN], BF16)
            zT1 = (zT1_0, zT1_1)
            zcT1 = singles.tile([2 * U, BN], BF16)
            nc.vector.memset(zT0[0:U, :], 0.0)
            nc.vector.memset(zT1_0[:, :], 0.0)
            nc.vector.memset(zT1_1[:, :], 0.0)

            def gconv_gate(l, zt):
                """val = sigmoid(z@(W0-W2) + S@(z@W1) + 2S^2@(z@W2) + bg)."""
                pqA = pq_pool.tile([N, 2, 4 * U], F32, tag="pq", name="pqA")
                pqB = pq_pool.tile([N, 2, 4 * U], F32, tag="pq", name="pqB")
                for b in range(BL):
                    dst = pqA[:, b, :] if b < 2 else pqB[:, b - 2, :]
                    nc.tensor.matmul(
                        dst,
                        lhsT=zt[:, b * N : (b + 1) * N],
                        rhs=w[l, "g12"],
                        start=True,
                        stop=True,
                    )
                q12 = sq_pool.tile([N, BL, 4 * U], BF16, tag="q12", name="q12")
                nc.vector.tensor_copy(q12[:, 0:2, :], pqA)
                nc.scalar.copy(q12[:, 2:4, :], pqB)

                pval = pval_pool.tile([2 * U, BN], F32, tag="pval", name="pval")
                nc.tensor.matmul(pval, lhsT=w[l, "g0"], rhs=zt, start=True, stop=False)
                for b in range(BL):
                    blk = pval[:, b * N : (b + 1) * N]
                    nc.tensor.matmul(
                        blk, lhsT=q12[:, b, 0 : 2 * U], rhs=st,
                        start=False, stop=False, skip_group_check=True,
                    )
                    nc.tensor.matmul(
                        blk, lhsT=q12[:, b, 2 * U : 4 * U], rhs=s2t,
                        start=False, stop=(b == BL - 1), skip_group_check=True,
                    )
                val = sval_pool.tile([2 * U, BN], BF16, tag="val", name="val")
                nc.scalar.activation(
                    val, pval, mybir.ActivationFunctionType.Sigmoid,
                    bias=bias[0 : 2 * U, 2 * l : 2 * l + 1],
                )
                # partition-0-aligned copy of u (TensorTensor operands must
                # share a start partition; this copy is off the critical path)
                u0 = sval_pool.tile([U, BN], BF16, tag="u0", name="u0")
                nc.gpsimd.tensor_copy(u0, val[U : 2 * U, :])
                return val, u0  # val = [r(0:U); u(U:2U)]

            def gconv_cand(l, zct):
                """c = tanh(zc@(W0-W2) + S@(zc@W1) + 2S^2@(zc@W2) + bc)."""
                pqc = pqc_pool.tile([N, BL, 2 * U], F32, tag="pqc", name="pqc")
                for b in range(BL):
                    nc.tensor.matmul(
                        pqc[:, b, :],
                        lhsT=zct[:, b * N : (b + 1) * N],
                        rhs=w[l, "c12"],
                        start=True,
                        stop=True,
                    )
                qc = sq_pool.tile([N, BL, 2 * U], BF16, tag="qc", name="qc")
                nc.vector.tensor_copy(qc[:, 0:2, :], pqc[:, 0:2, :])
                nc.scalar.copy(qc[:, 2:4, :], pqc[:, 2:4, :])

                pc = pc_pool.tile([U, BN], F32, tag="pc", name="pc")
                nc.tensor.matmul(pc, lhsT=w[l, "c0"], rhs=zct, start=True, stop=False)
                for b in range(BL):
                    blk = pc[:, b * N : (b + 1) * N]
                    nc.tensor.matmul(
                        blk, lhsT=qc[:, b, 0:U], rhs=st,
                        start=False, stop=False, skip_group_check=True,
                    )
                    nc.tensor.matmul(
                        blk, lhsT=qc[:, b, U : 2 * U], rhs=s2t,
                        start=False, stop=(b == BL - 1), skip_group_check=True,
                    )
                c = sval_pool.tile([U, BN], BF16, tag="c", name="c")
                nc.scalar.activation(
                    c, pc, mybir.ActivationFunctionType.Tanh,
                    bias=bias[0:U, 2 * l + 1 : 2 * l + 2],
                )
                return c

            def update(u0, c, h_prev, h_out):
                """h_out = c + u*(h_prev - c)."""
                d = sval_pool.tile([U, BN], BF16, tag="d", name="d")
                nc.vector.tensor_sub(d, h_prev, c)
                ud = sval_pool.tile([U, BN], BF16, tag="ud", name="ud")
                nc.vector.tensor_mul(ud, u0, d)
                nc.vector.tensor_add(h_out, ud, c)

            for t in range(t_steps):
                cur, nxt = t % 2, (t + 1) % 2
                nc.sync.dma_start(out=zT0[U : U + DIN, :], in_=xT_e[t])
                nc.sync.dma_start(out=zcT0[U : U + DIN, :], in_=xT_e[t])

                # ---- layer 0 ----
                val0, u0_0 = gconv_gate(0, zT0)
                nc.vector.tensor_mul(zcT0[0:U, :], val0[0:U, :], zT0[0:U, :])
                c0 = gconv_cand(0, zcT0)
                update(u0_0, c0, zT0[0:U, :], zT0[0:U, :])

                # h1 fanout into layer1's z and zc (h1 lives at partitions U:2U)
                nc.vector.tensor_copy(zT1[cur][U : 2 * U, :], zT0[0:U, :])
                nc.gpsimd.tensor_copy(zcT1[U : 2 * U, :], zT0[0:U, :])

                # ---- layer 1 ----  (z = [h2(0:U); h1(U:2U)])
                val1, u0_1 = gconv_gate(1, zT1[cur])
                nc.vector.tensor_mul(
                    zcT1[0:U, :], val1[0:U, :], zT1[cur][0:U, :]
                )
                c1 = gconv_cand(1, zcT1)
                update(u0_1, c1, zT1[cur][0:U, :], zT1[nxt][0:U, :])
                nc.sync.dma_start(out=h2seq_e[t], in_=zT1[nxt][0:U, :])

    nc.compile()
    return nc


def _prep_shared(support, W0_gate, W0_cand, W1_gate, W1_cand,
                 b0_gate, b0_cand, b1_gate, b1_cand):
    f = np.float32
    S = np.asarray(support, f)
    seg = {
        "S_T": np.ascontiguousarray(S.T),
        "S2_T": np.ascontiguousarray((2.0 * (S @ S)).T),
    }
    bias = np.zeros((N, 4), f)
    for l, (Wg, Wc, bg, bc) in enumerate(
        ((W0_gate, W0_cand, b0_gate, b0_cand), (W1_gate, W1_cand, b1_gate, b1_cand))
    ):
        Wg = np.asarray(Wg, f)
        Wc = np.asarray(Wc, f)
        g = [Wg[m::3] for m in range(3)]
        c = [Wc[m::3] for m in range(3)]
        if l == 0:
            # device z-layout for layer0 is [h(64); x(16)]
            perm = np.concatenate([np.arange(DIN, DIN + U), np.arange(DIN)])
        else:
            # device z-layout for layer1 is [h2(64); h1(64)]
            perm = np.concatenate([np.arange(U, 2 * U), np.arange(U)])
        g = [gm[perm] for gm in g]
        c = [cm[perm] for cm in c]
        seg[(l, "g12")] = np.concatenate([g[1], g[2]], axis=1)
        seg[(l, "g0")] = g[0] - g[2]
        seg[(l, "c12")] = np.concatenate([c[1], c[2]], axis=1)
        seg[(l, "c0")] = c[0] - c[2]
        bias[0 : 2 * U, 2 * l] = np.asarray(bg, f).reshape(-1)
        bias[0:U, 2 * l + 1] = np.asarray(bc, f).reshape(-1)
    blob = np.zeros((N, BLOB_COLS), BF16_NP)
    for key, (rows, c0, cols) in _BLOB_LAYOUT.items():
        a = seg[key]
        assert a.shape == (rows, cols), (key, a.shape, rows, cols)
        blob[:rows, c0 : c0 + cols] = a.astype(BF16_NP)
    return {"blob": blob, "bias": bias}


def run_cores(inputs, t_steps=T, trace=False):
    """Build in_maps, run the SPMD kernel, return per-core h2 sequences."""
    input_seq = np.asarray(inputs["input_seq"], np.float32)
    shared = _prep_shared(
        inputs["support"], inputs["W0_gate"], inputs["W0_cand"],
        inputs["W1_gate"], inputs["W1_cand"],
        inputs["b0_gate"], inputs["b0_cand"], inputs["b1_gate"], inputs["b1_cand"],
    )
    in_maps = []
    for k in range(NCORES):
        xs = input_seq[k * BL : (k + 1) * BL, :t_steps]  # (BL, t, N, DIN)
        xT = np.ascontiguousarray(
            np.transpose(xs, (1, 3, 0, 2)).reshape(t_steps, DIN, BN)
        ).astype(BF16_NP)
        in_maps.append(dict(shared, xT=xT))
    if t_steps not in _NC_CACHE:
        _NC_CACHE[t_steps] = _build_nc(t_steps)
    nc = _NC_CACHE[t_steps]
    res = run_bass_kernel_spmd(nc, in_maps, list(range(NCORES)), trace=trace)
    return res


def finish_host(results, inputs):
    """Host tail: pick h2 at t=seq_len-1, then relu -> fc -> node max-pool."""
    W_fc = np.asarray(inputs["W_fc"], np.float32)
    b_fc = np.asarray(inputs["b_fc"], np.float32)
    seq = np.asarray(inputs["seq_lengths"]).astype(np.int64)
    out = np.empty((B, C), np.float32)
    for k in range(NCORES):
        h2seq = results[k]["h2seq"]  # (t_steps, U, BN) bf16
        t_steps = h2seq.shape[0]
        for b in range(BL):
            tb = int(min(seq[k * BL + b] - 1, t_steps - 1))
            blk = np.asarray(h2seq[tb, :, b * N : (b + 1) * N], np.float32).T  # (N, U)
            logits = np.maximum(blk, 0.0) @ W_fc + b_fc  # (N, C)
            out[k * BL + b] = logits.max(axis=0)
    return out


def kernel(**inputs):
    seq = np.asarray(inputs["seq_lengths"]).astype(np.int64)
    t_steps = int(min(T, max(1, int(seq.max()))))
    res = run_cores(inputs, t_steps=t_steps)
    return finish_host(res.results, inputs)
